# revision 17
# baseline (speedup 1.0000x reference)
"""Multi-head self-attention Trainium2 kernel (8 NeuronCores).

Sharding: 2 groups x 4 cores. Group g owns batch element b=g (data
parallel); within a group each core owns 4 of the 16 heads (tensor
parallel over the QKV projections). A single intra-group AllGather
collects the full attention output, after which each core computes a
256-row slice of out.T (its host-provided column slice of wo) over all
tokens - the program stays rank-agnostic, only input data differs.

All device-side tensors live in transposed [feature, token] layout so
every matmul contracts over the partition dimension. Softmax runs on
transposed scores [t_k, t_q]: the key mask becomes a per-partition bias
fused into the Exp activation, and the denominator is a ones-vector
matmul (column reduction over partitions).

The host compacts key/value tokens to the unmasked subset (padded to a
multiple of 128) before sharding - masked keys contribute exactly 0
after softmax, so dropping them is lossless and halves attention work.
"""

import sys

sys.path.insert(0, "/opt/trn_rl_repo")

from contextlib import ExitStack

import ml_dtypes
import numpy as np

import concourse.bass as bass
import concourse.mybir as mybir
import concourse.tile as tile
from concourse import bacc
from concourse.masks import make_identity

B, S, D, H = 2, 2048, 1024, 16
DK = D // H  # 64
NCORES = 8
RPG = 4  # ranks per group (tensor-parallel width)
GROUPS = NCORES // RPG
HPC = H // RPG  # heads per core = 4
DSL = HPC * DK  # per-core slice of d_model = 256
TQME = S // RPG  # per-core token slice for the output projection = 512

F32 = mybir.dt.float32
BF16 = mybir.dt.bfloat16
F32R = mybir.dt.float32r
BF16_NP = ml_dtypes.bfloat16

NEG = -1e9


def _ceil_mult(x, m):
    return ((x + m - 1) // m) * m


def build_mha(k_keep: int) -> bass.Bass:
    """Build the SPMD Bass program (identical on all 8 cores)."""
    assert k_keep % 128 == 0
    KT = k_keep // 128  # key-token tiles
    KQT = 2  # dout tiles per core (DSL/128)
    nc = bacc.Bacc(None, target_bir_lowering=False, num_devices=NCORES)

    xq = nc.declare_dram_parameter("xq", [D, S], BF16, isOutput=False)
    xk = nc.declare_dram_parameter("xk", [D, k_keep], BF16, isOutput=False)
    xv = nc.declare_dram_parameter("xv", [D, k_keep], BF16, isOutput=False)
    wqT = nc.declare_dram_parameter("wqT", [D, DSL], BF16, isOutput=False)
    wkT = nc.declare_dram_parameter("wkT", [D, DSL], BF16, isOutput=False)
    wvT = nc.declare_dram_parameter("wvT", [D, DSL], BF16, isOutput=False)
    woT = nc.declare_dram_parameter("woT", [D, DSL], BF16, isOutput=False)
    bqkv = nc.declare_dram_parameter("bqkv", [128, 6], F32, isOutput=False)
    bob = nc.declare_dram_parameter("bob", [128, 2], F32, isOutput=False)
    mbias = nc.declare_dram_parameter("mbias", [128, KT], F32, isOutput=False)
    out = nc.declare_dram_parameter("out", [DSL, S], F32, isOutput=True)

    den_dt = F32

    with tile.TileContext(nc) as tc, ExitStack() as ctx:
        const = ctx.enter_context(tc.tile_pool(name="const", bufs=1))
        xpool = ctx.enter_context(tc.tile_pool(name="xpool", bufs=3))
        actsb = ctx.enter_context(tc.tile_pool(name="actsb", bufs=1))
        ppool = ctx.enter_context(tc.tile_pool(name="ppool", bufs=3))
        dpool = ctx.enter_context(tc.tile_pool(name="dpool", bufs=2))
        opool = ctx.enter_context(tc.tile_pool(name="opool", bufs=3))
        dram = ctx.enter_context(tc.tile_pool(name="dram", bufs=2, space="DRAM"))

        pp_mm = ctx.enter_context(tc.tile_pool(name="pp_mm", bufs=2, space="PSUM"))
        pp_sc = ctx.enter_context(tc.tile_pool(name="pp_sc", bufs=1, space="PSUM"))
        pp_x = ctx.enter_context(tc.tile_pool(name="pp_x", bufs=1, space="PSUM"))

        # ---- constants / weights ----
        wq_sb = const.tile([128, 8, DSL], BF16, tag="wq")
        wk_sb = const.tile([128, 8, DSL], BF16, tag="wk")
        wv_sb = const.tile([128, 8, DSL], BF16, tag="wv")
        wo_sb = const.tile([128, 8, DSL], BF16, tag="wo")
        nc.sync.dma_start(wq_sb[:], wqT.rearrange("(ko p) m -> p ko m", p=128))
        nc.sync.dma_start(wk_sb[:], wkT.rearrange("(ko p) m -> p ko m", p=128))
        nc.sync.dma_start(wv_sb[:], wvT.rearrange("(ko p) m -> p ko m", p=128))
        nc.sync.dma_start(wo_sb[:], woT.rearrange("(ko p) m -> p ko m", p=128))
        bqkv_sb = const.tile([128, 6], F32, tag="bqkv")
        bob_sb = const.tile([128, 2], F32, tag="bob")
        mb_sb = const.tile([128, KT], F32, tag="mb")
        nc.sync.dma_start(bqkv_sb[:], bqkv[:])
        nc.sync.dma_start(bob_sb[:], bob[:])
        nc.sync.dma_start(mb_sb[:], mbias[:])
        ident = const.tile([128, 128], BF16, tag="ident")
        make_identity(nc, ident[:])
        ones128 = const.tile([128, 1], BF16, tag="ones128")
        nc.gpsimd.memset(ones128[:], 1.0)

        # ---- projections: out.T[dout, t] = W_slice @ x.T ----
        qT_sb = actsb.tile([128, KQT, S], BF16, tag="qT")
        kT_sb = actsb.tile([128, KQT, k_keep], BF16, tag="kT")
        vT_sb = actsb.tile([128, KQT, k_keep], BF16, tag="vT")

        def project(xparam, w_sb, dst_sb, t_total, bcol0):
            nchunks = (t_total + 511) // 512
            for c in range(nchunks):
                t0 = c * 512
                tw = min(512, t_total - t0)
                xt = xpool.tile([128, 8, 512], BF16, tag="xT")
                nc.sync.dma_start(
                    xt[:, :, :tw],
                    xparam[:, t0 : t0 + tw].rearrange("(ko p) t -> p ko t", p=128),
                )
                for dt in range(KQT):
                    ps = pp_mm.tile([128, 512], F32, tag="mm")
                    for ko in range(8):
                        nc.tensor.matmul(
                            ps[:, :tw],
                            lhsT=w_sb[:, ko, dt * 128 : (dt + 1) * 128],
                            rhs=xt[:, ko, :tw],
                            start=(ko == 0),
                            stop=(ko == 7),
                        )
                    nc.vector.tensor_add(
                        dst_sb[:, dt, t0 : t0 + tw],
                        ps[:, :tw],
                        bqkv_sb[:, bcol0 + dt, None].to_broadcast((128, tw)),
                    )

        project(xk, wk_sb, kT_sb, k_keep, 2)
        project(xq, wq_sb, qT_sb, S, 0)
        project(xv, wv_sb, vT_sb, k_keep, 4)

        # ---- v natural layout [t_k, dv] via PE transpose ----
        v_nat = actsb.tile([128, KT, DSL], BF16, tag="v_nat")
        for tk in range(KT):
            for dt in range(KQT):
                pt = pp_mm.tile([128, 128], BF16, tag="mm")
                nc.tensor.transpose(
                    pt[:], vT_sb[:, dt, tk * 128 : (tk + 1) * 128], ident[:]
                )
                nc.vector.tensor_copy(v_nat[:, tk, dt * 128 : (dt + 1) * 128], pt[:])

        # ---- attention (scores.T layout), heads in row/col-packed pairs ----
        xT_local = actsb.tile([128, KQT, S], BF16, tag="xT_local")
        for hp in range(KQT):  # head-pair index (= dout tile)
            c0 = hp * 128
            for tqc in range(2):  # t_q chunks of 1024
                q0 = tqc * 1024
                xacc = dpool.tile([128, 1024], F32, tag="xacc")
                denA = dpool.tile([128, 1024], den_dt, tag="denA")
                denB = dpool.tile([128, 1024], den_dt, tag="denB")
                for tk in range(KT):
                    scA = pp_sc.tile([128, 1024], F32, tag="scA")
                    scB = pp_sc.tile([128, 1024], F32, tag="scB")
                    for sub in range(2):
                        s0 = sub * 512
                        nc.tensor.matmul(
                            scA[:, s0 : s0 + 512],
                            lhsT=kT_sb[0:64, hp, tk * 128 : (tk + 1) * 128],
                            rhs=qT_sb[0:64, hp, q0 + s0 : q0 + s0 + 512],
                            start=True,
                            stop=True,
                            tile_position=(0, 0),
                        )
                        nc.tensor.matmul(
                            scB[:, s0 : s0 + 512],
                            lhsT=kT_sb[64:128, hp, tk * 128 : (tk + 1) * 128],
                            rhs=qT_sb[64:128, hp, q0 + s0 : q0 + s0 + 512],
                            start=True,
                            stop=True,
                            tile_position=(64, 0),
                        )
                    pA = ppool.tile([128, 1024], BF16, tag="pA")
                    pB = ppool.tile([128, 1024], BF16, tag="pB")
                    nc.scalar.activation(
                        pA[:], scA[:], mybir.ActivationFunctionType.Exp,
                        bias=mb_sb[:, tk, None], scale=0.125,
                    )
                    nc.scalar.activation(
                        pB[:], scB[:], mybir.ActivationFunctionType.Exp,
                        bias=mb_sb[:, tk, None], scale=0.125,
                    )
                    if tk == 0:
                        nc.vector.tensor_copy(denA[:], pA[:])
                        nc.vector.tensor_copy(denB[:], pB[:])
                    else:
                        nc.vector.tensor_add(denA[:], denA[:], pA[:])
                        nc.vector.tensor_add(denB[:], denB[:], pB[:])
                    av = pp_x.tile([128, 1024], F32, tag="av")
                    for sub in range(2):
                        s0 = sub * 512
                        nc.tensor.matmul(
                            av[0:64, s0 : s0 + 512],
                            lhsT=v_nat[:, tk, c0 : c0 + 64],
                            rhs=pA[:, s0 : s0 + 512],
                            start=True,
                            stop=True,
                            tile_position=(0, 0),
                        )
                        nc.tensor.matmul(
                            av[64:128, s0 : s0 + 512],
                            lhsT=v_nat[:, tk, c0 + 64 : c0 + 128],
                            rhs=pB[:, s0 : s0 + 512],
                            start=True,
                            stop=True,
                            tile_position=(0, 64),
                        )
                    if tk == 0:
                        nc.vector.tensor_copy(xacc[:], av[:])
                    else:
                        nc.vector.tensor_add(xacc[:], xacc[:], av[:])
                # normalize: column-sum denominators, reciprocal, broadcast
                for sub in range(2):
                    s0 = sub * 512
                    pbs = dpool.tile([128, 512], F32, tag="pbs")
                    for hb, den in ((0, denA), (1, denB)):
                        denb = dpool.tile([128, 512], BF16, tag="denb")
                        nc.vector.tensor_copy(denb[:], den[:, s0 : s0 + 512])
                        pd = pp_mm.tile([1, 512], F32, tag="mm")
                        nc.tensor.matmul(
                            pd[:], lhsT=ones128[:], rhs=denb[:],
                            start=True, stop=True,
                        )
                        recd = dpool.tile([1, 512], F32, tag=f"recd{hb}")
                        nc.vector.reciprocal(recd[:], pd[:])
                        rdd = dram.tile([1, 512], F32, tag=f"rdd{hb}")
                        nc.sync.dma_start(rdd[:], recd[:])
                        nc.sync.dma_start(
                            pbs[hb * 64 : (hb + 1) * 64, :],
                            rdd[:].to_broadcast((64, 512)),
                        )
                    nc.vector.tensor_mul(
                        xT_local[:, hp, q0 + s0 : q0 + s0 + 512],
                        xacc[:, s0 : s0 + 512],
                        pbs[:],
                    )

        # ---- AllGather within the 4-core group: full x.T on every core ----
        ag_in = dram.tile([DSL, S], BF16, tag="ag_in")
        ag_out = dram.tile([D, S], BF16, tag="ag_out")
        nc.sync.dma_start(
            ag_in[:].rearrange("(hp p) t -> p hp t", p=128),
            xT_local[:],
        )
        nc.gpsimd.collective_compute(
            "AllGather",
            mybir.AluOpType.bypass,
            replica_groups=[[0, 1, 2, 3], [4, 5, 6, 7]],
            ins=[ag_in[:]],
            outs=[ag_out[:]],
        )
        xme_sb = actsb.tile([128, 8, S], BF16, tag="xme")
        nc.sync.dma_start(xme_sb[:], ag_out[:].rearrange("(ko p) t -> p ko t", p=128))

        # ---- output projection: out.T[my dout slice, :] = wo_sl @ x.T + bo ----
        for dt in range(KQT):
            for c in range(4):
                t0 = c * 512
                po = pp_mm.tile([128, 512], F32, tag="mm")
                for ko in range(8):
                    nc.tensor.matmul(
                        po[:],
                        lhsT=wo_sb[:, ko, dt * 128 : (dt + 1) * 128],
                        rhs=xme_sb[:, ko, t0 : t0 + 512],
                        start=(ko == 0),
                        stop=(ko == 7),
                    )
                ot = opool.tile([128, 512], F32, tag="ot")
                nc.vector.tensor_add(
                    ot[:], po[:], bob_sb[:, dt, None].to_broadcast((128, 512))
                )
                nc.sync.dma_start(out[dt * 128 : (dt + 1) * 128, t0 : t0 + 512], ot[:])

    nc.compile()
    return nc


def prep_inputs(query, key, value, mask, wq, bq, wk, bk, wv, bv, wo, bo):
    """Compact masked keys, shard, and lay out per-core input maps."""
    keep = [np.nonzero(np.asarray(mask[b]) != 0)[0] for b in range(B)]
    k_keep = max(128, _ceil_mult(max(len(i) for i in keep), 128))
    k_keep = min(k_keep, S)
    if max(len(i) for i in keep) > k_keep:  # cannot happen, but be safe
        k_keep = S

    in_maps = []
    for g in range(GROUPS):
        idx = keep[g]
        nk = len(idx)
        xq_g = np.ascontiguousarray(np.asarray(query[g]).T).astype(BF16_NP)
        xk_g = np.zeros((D, k_keep), BF16_NP)
        xv_g = np.zeros((D, k_keep), BF16_NP)
        xk_g[:, :nk] = np.asarray(key[g]).T[:, idx].astype(BF16_NP)
        xv_g[:, :nk] = np.asarray(value[g]).T[:, idx].astype(BF16_NP)
        mb = np.full(k_keep, NEG, np.float32)
        mb[:nk] = 0.0
        mb = np.ascontiguousarray(mb.reshape(k_keep // 128, 128).T)
        for r in range(RPG):
            sl = slice(DSL * r, DSL * (r + 1))
            bq_r = np.asarray(bq)[sl].reshape(2, 128)
            bk_r = np.asarray(bk)[sl].reshape(2, 128)
            bv_r = np.asarray(bv)[sl].reshape(2, 128)
            bqkv = np.stack(
                [bq_r[0], bq_r[1], bk_r[0], bk_r[1], bv_r[0], bv_r[1]], axis=1
            ).astype(np.float32)
            bob_r = np.ascontiguousarray(
                np.asarray(bo)[sl].reshape(2, 128).T
            ).astype(np.float32)
            in_maps.append(
                {
                    "xq": xq_g,
                    "xk": xk_g,
                    "xv": xv_g,
                    "wqT": np.ascontiguousarray(np.asarray(wq)[sl].T).astype(BF16_NP),
                    "wkT": np.ascontiguousarray(np.asarray(wk)[sl].T).astype(BF16_NP),
                    "wvT": np.ascontiguousarray(np.asarray(wv)[sl].T).astype(BF16_NP),
                    "woT": np.ascontiguousarray(np.asarray(wo)[sl].T).astype(BF16_NP),
                    "bqkv": np.ascontiguousarray(bqkv),
                    "bob": bob_r,
                    "mbias": mb.astype(np.float32),
                }
            )
    return in_maps, k_keep


def assemble_output(results):
    """results: list of 8 dicts with 'out' [DSL, S] = out.T row slices."""
    full = np.empty((B, S, D), np.float32)
    for g in range(GROUPS):
        outT = np.concatenate(
            [results[RPG * g + r]["out"] for r in range(RPG)], axis=0
        )  # [D, S]
        full[g] = outT.T
    return full


_BUILT = {}


def _get_program(k_keep):
    if k_keep not in _BUILT:
        _BUILT[k_keep] = build_mha(k_keep)
    return _BUILT[k_keep]


def _run(inputs, trace=False):
    from concourse.bass_utils import run_bass_kernel_spmd

    in_maps, k_keep = prep_inputs(**inputs)
    nc = _get_program(k_keep)
    res = run_bass_kernel_spmd(
        nc, in_maps, list(range(NCORES)), trace=trace
    )
    out = assemble_output(res.results)
    return out, res


def kernel(**inputs) -> np.ndarray:
    out, _ = _run(inputs, trace=False)
    return out


if __name__ == "__main__":
    mode = sys.argv[1] if len(sys.argv) > 1 else "build"
    if mode == "build":
        nc = build_mha(int(sys.argv[2]) if len(sys.argv) > 2 else 1152)
        n_inst = sum(len(bb.instructions) for bb in nc.main_func.blocks)
        print(f"build ok, {n_inst} instructions")
